# revision 5
# baseline (speedup 1.0000x reference)
"""KAN block (2x KAN layer, dense_mlp) TRN2 Bass kernel — data-parallel on 8 cores.

Full inputs in, full output out. Tokens (B*S = 4096) are sharded 8 ways
(512 per core); weights are replicated.

Device math per KAN layer (out = silu(x) @ wb.T + einsum('nig,oig->no', B(x), ws)):
the 8 cubic B-spline bases B_g on the uniform 12-knot grid are expressed
exactly through 12 truncated-cube features

    a_j = relu(t_j - |x|)   (j = 6..11; knots t_0..t_5 < 0 never activate on |x|)
    u_j = a_j^3 * [x >= 0],   v_j = a_j^3 * [x < 0]

via B_g = sum_j Mu[g,j] u_j + Mv[g,j] v_j  (truncated-power representation of
B-splines evaluated from the near side, so feature magnitudes stay <= 2.2^3;
x outside the grid yields exact zeros on the feature side). The 8->12 map is
folded into the spline weights on the host, making each layer ONE dense
matmul with contraction over 13*I (silu + 12 spline features per input dim),
executed in fp32r (FP22 mantissa) at full PE speed with fp32 PSUM accumulation.

Layout: activations transposed (d on partitions, tokens on free dim), so
feature generation is elementwise on [128, 512] tiles and matmuls are
[128k x 128m]^T @ [128k x 512tok] -> PSUM [128m x 512tok].
"""

import numpy as np
from contextlib import ExitStack
from math import comb

import concourse.bass as bass
import concourse.bacc as bacc
import concourse.mybir as mybir
import concourse.tile as tile
from concourse.bass_utils import run_bass_kernel_spmd

F32 = mybir.dt.float32
F32R = mybir.dt.float32r
AF = mybir.ActivationFunctionType
ALU = mybir.AluOpType

# Problem constants (hardcoded per contract)
B, S, D, F = 2, 2048, 512, 2048
N_CORES = 8
T = (B * S) // N_CORES          # 512 tokens per core
G_INT, K_ORD = 5, 3
NKNOT = 12
NFEAT = 13                      # [silu, u6..u11, v6..v11]
ACT_CHAINS = (6, 7, 8, 9, 10)   # relu+square on ScalarE
DVE_CHAINS = (11,)              # full chain on VectorE (produces -a^3)
G1 = 4                          # layer-1 output tiles per PSUM group


def knots_f32():
    return (np.arange(-K_ORD, G_INT + K_ORD + 1, dtype=np.float32)
            * np.float32(2.0 / G_INT) - np.float32(1.0))


def fold_maps():
    inv6h3 = 1.0 / (6.0 * (2.0 / G_INT) ** 3)
    M = np.zeros((8, NKNOT))
    for g in range(8):
        for k in range(5):
            M[g, g + k] = ((-1) ** k) * comb(4, k) * inv6h3
    return M[:, 6:12].copy(), M[:, 5::-1].copy()


def fold_weights(wb, ws):
    """wb: (O, I), ws: (O, I, 8) -> (O, I, 13) fp32 augmented weights."""
    Mu, Mv = fold_maps()
    Wu = np.einsum('oig,gj->oij', ws.astype(np.float64), Mu)
    Wv = np.einsum('oig,gj->oij', ws.astype(np.float64), Mv)
    for j in DVE_CHAINS:
        Wu[:, :, j - 6] *= -1.0
        Wv[:, :, j - 6] *= -1.0
    Waug = np.concatenate([wb.astype(np.float64)[:, :, None], Wu, Wv], axis=2)
    return np.ascontiguousarray(Waug.astype(np.float32))


def pack_w1(Waug1):
    """(F, D, 13) -> (NG1, D_T*13, 128, G1*128): [mgroup, ktile, k_part, m_free]."""
    D_T, F_T = D // 128, F // 128
    NG1 = F_T // G1
    A = Waug1.reshape(NG1, G1 * 128, D_T, 128, NFEAT)
    A = A.transpose(0, 2, 4, 3, 1)
    return np.ascontiguousarray(A.reshape(NG1, D_T * NFEAT, 128, G1 * 128))


def pack_w2(Waug2):
    """(D, F, 13) -> (F_T, 128, 13, D): [d2group, k_part, feature, m_free]."""
    F_T = F // 128
    return np.ascontiguousarray(Waug2.transpose(1, 2, 0).reshape(F_T, 128, NFEAT, D))


def build_kernel():
    D_T, F_T = D // 128, F // 128
    NG1 = F_T // G1
    KT1, KT2 = D_T * NFEAT, F_T * NFEAT
    t = knots_f32()

    nc = bacc.Bacc()

    # knot constants as [128,1] const APs (activation bias operands)
    for j in range(6, 12):
        val = float(t[j])
        ctens = nc.alloc_sbuf_tensor(f"const-knot-{j}", [128, 1], F32)
        nc.gpsimd.memset(ctens.ap(), val)
        nc.const_aps.aps[(F32, val)] = ctens.ap()
    nc.all_engine_barrier()

    xT = nc.declare_dram_parameter("xT", [D, T], F32, isOutput=False)
    w1t = nc.declare_dram_parameter("w1t", [NG1, KT1, 128, G1 * 128], F32R,
                                    isOutput=False)
    w2t = nc.declare_dram_parameter("w2t", [F_T, 128, NFEAT, D], F32R,
                                    isOutput=False)
    outT = nc.declare_dram_parameter("outT", [D, T], F32, isOutput=True)

    with ExitStack() as ctx:
        tc = ctx.enter_context(tile.TileContext(nc))
        xpool = ctx.enter_context(tc.tile_pool(name="xp", bufs=1))
        f1pool = ctx.enter_context(tc.tile_pool(name="f1p", bufs=20))
        f2pool = ctx.enter_context(tc.tile_pool(name="f2p", bufs=26))
        scr = ctx.enter_context(tc.tile_pool(name="scr", bufs=3))
        w1pool = ctx.enter_context(tc.tile_pool(name="w1p", bufs=3))
        w2pool = ctx.enter_context(tc.tile_pool(name="w2p", bufs=2))
        opool = ctx.enter_context(tc.tile_pool(name="op", bufs=2))
        pp = ctx.enter_context(tc.tile_pool(name="pp", bufs=1, space="PSUM"))

        xtiles = []
        for dt in range(D_T):
            xt = xpool.tile([128, T], F32, name=f"x{dt}", tag=f"x{dt}")
            nc.sync.dma_start(out=xt, in_=xT[dt * 128:(dt + 1) * 128, :])
            xtiles.append(xt)

        psum2 = [pp.tile([128, T], F32, name=f"ps2_{m}", tag=f"l2psum{m}")
                 for m in range(D_T)]

        def gen_features(src, pool, blk):
            sig = scr.tile([128, T], F32, name=f"sig{blk}", tag="sig")
            nc.scalar.activation(sig, src, AF.Sigmoid)
            sil = pool.tile([128, T], F32R, name=f"sil{blk}", tag="feat")
            nc.vector.tensor_mul(sil, src, sig)
            y = scr.tile([128, T], F32, name=f"y{blk}", tag="y")
            nc.scalar.activation(y, src, AF.Abs)
            pos = scr.tile([128, T], F32, name=f"pos{blk}", tag="pos")
            nc.vector.tensor_scalar(out=pos, in0=src, scalar1=0.0, scalar2=None,
                                    op0=ALU.is_ge)
            us, vs = [], []
            for j in range(6, 12):
                tj = float(t[j])
                a = scr.tile([128, T], F32, name=f"a{blk}_{j}", tag="a")
                q = scr.tile([128, T], F32, name=f"q{blk}_{j}", tag="q")
                if j in ACT_CHAINS:
                    nc.scalar.activation(a, y, AF.Relu, bias=tj, scale=-1.0)
                    nc.scalar.activation(q, a, AF.Square)
                else:   # DVE chain: a = min(y - tj, 0) = -relu(tj - y)
                    nc.vector.tensor_scalar(out=a, in0=y, scalar1=tj,
                                            scalar2=0.0, op0=ALU.subtract,
                                            op1=ALU.min)
                    nc.vector.tensor_mul(q, a, a)
                c = scr.tile([128, T], F32, name=f"c{blk}_{j}", tag="c", bufs=4)
                nc.vector.tensor_mul(c, q, a)
                u = pool.tile([128, T], F32R, name=f"u{blk}_{j}", tag="feat")
                nc.vector.tensor_mul(u, c, pos)
                v = pool.tile([128, T], F32R, name=f"v{blk}_{j}", tag="feat")
                nc.gpsimd.tensor_sub(v, c, u)
                us.append(u)
                vs.append(v)
            return [sil] + us + vs

        for gm in range(NG1):
            psum1 = [pp.tile([128, T], F32, name=f"ps1_{gm}_{mi}",
                             tag=f"l1psum{mi}") for mi in range(G1)]
            for dt in range(D_T):
                feats = gen_features(xtiles[dt], f1pool, blk=f"a{gm}d{dt}")
                for f in range(NFEAT):
                    kt = dt * NFEAT + f
                    wt = w1pool.tile([128, G1 * 128], F32R,
                                     name=f"w1_{gm}_{kt}", tag="w1")
                    nc.sync.dma_start(out=wt, in_=w1t[gm, kt, :, :])
                    for mi in range(G1):
                        nc.tensor.matmul(
                            psum1[mi],
                            lhsT=wt[:, mi * 128:(mi + 1) * 128],
                            rhs=feats[f],
                            start=(kt == 0), stop=(kt == KT1 - 1),
                        )
            for mi in range(G1):
                g2 = gm * G1 + mi
                l2f = gen_features(psum1[mi], f2pool, blk=f"b{g2}")
                wt2a = w2pool.tile([128, 7, D], F32R, name=f"w2a_{g2}", tag="w2h")
                nc.sync.dma_start(out=wt2a, in_=w2t[g2, :, 0:7, :])
                wt2b = w2pool.tile([128, 6, D], F32R, name=f"w2b_{g2}", tag="w2h")
                nc.sync.dma_start(out=wt2b, in_=w2t[g2, :, 7:13, :])
                for f in range(NFEAT):
                    kt2 = g2 * NFEAT + f
                    wsl = wt2a[:, f, :] if f < 7 else wt2b[:, f - 7, :]
                    for m2 in range(D_T):
                        nc.tensor.matmul(
                            psum2[m2],
                            lhsT=wsl[:, m2 * 128:(m2 + 1) * 128],
                            rhs=l2f[f],
                            start=(kt2 == 0), stop=(kt2 == KT2 - 1),
                        )
        for m2 in range(D_T):
            ot = opool.tile([128, T], F32, name=f"o{m2}", tag="out")
            nc.scalar.activation(ot, psum2[m2], AF.Copy)
            nc.sync.dma_start(out=outT[m2 * 128:(m2 + 1) * 128, :], in_=ot)

    nc.finalize()
    return nc


_NC_CACHE = None


def _get_nc():
    global _NC_CACHE
    if _NC_CACHE is None:
        _NC_CACHE = build_kernel()
    return _NC_CACHE


def run(x, w1_base, w1_spline, w2_base, w2_spline, trace=False, **spmd_kwargs):
    x = np.asarray(x, dtype=np.float32)
    xf = np.ascontiguousarray(x.reshape(B * S, D))
    w1p = pack_w1(fold_weights(np.asarray(w1_base), np.asarray(w1_spline)))
    w2p = pack_w2(fold_weights(np.asarray(w2_base), np.asarray(w2_spline)))
    in_maps = []
    for c in range(N_CORES):
        shard = xf[c * T:(c + 1) * T]
        in_maps.append({
            "xT": np.ascontiguousarray(shard.T),
            "w1t": w1p,
            "w2t": w2p,
        })
    nc = _get_nc()
    res = run_bass_kernel_spmd(nc, in_maps, list(range(N_CORES)),
                               trace=trace, **spmd_kwargs)
    outs = [np.asarray(r["outT"]).T for r in res.results]   # each (T, D)
    out = np.concatenate(outs, axis=0).reshape(B, S, D).astype(np.float32)
    return out, res


def kernel(x, grid, w1_base, w1_spline, w2_base, w2_spline):
    out, _ = run(x, w1_base, w1_spline, w2_base, w2_spline)
    return out
